# revision 1
# baseline (speedup 1.0000x reference)
"""GCN teacher 3-layer SpMM kernel for 8 trn2 NeuronCores (self-contained).

Strategy: row-shard nodes across 8 cores (147 tiles x 128 rows each, padded,
with a degree-balancing row permutation); per layer, per-edge dma_gather of
h[col] rows (int16 range-relative slot indices over 5 sub-32768-slot ranges),
scatter-add via one-hot PE matmuls (128-edge chunks into 32-row windows, fp8
one-hot from host scaled by vals on DVE), PSUM accumulation per window,
AllGather republishes h between layers. All fp32 compute.
"""
import sys as _sys
for _p in ("/opt/trn_rl_repo",):
    if _p not in _sys.path:
        _sys.path.insert(0, _p)

import math
import numpy as np
import ml_dtypes

import concourse.bass as bass
import concourse.bacc as bacc
import concourse.tile as tile
from concourse import mybir

F32 = mybir.dt.float32
I16 = mybir.dt.int16
F8 = mybir.dt.float8e4
F8NP = ml_dtypes.float8_e4m3

D = 64
WPT = 4          # 32-row windows per 128-row tile
NR = 5           # source ranges


def _chunk_layout(T, TB, B):
    """Static call/chunk layout tables shared by prep() and build()."""
    NCHT = WPT * sum(B)
    soff = [WPT * sum(B[:r]) for r in range(NR)]  # in-tile slot offset per range
    batches = []
    t0 = 0
    while t0 < T:
        tb = min(TB, T - t0)
        batches.append((t0, tb))
        t0 += tb
    return NCHT, soff, batches


def prep(n_cores, T, n_nodes, ego, vals, rows, cols, TB=4):
    N_pad = n_cores * T * 128
    BLK = T * 128
    nwin = N_pad // 32
    E = rows.shape[0]
    range_len = math.ceil(N_pad / NR)
    assert range_len <= 32768

    rows = np.asarray(rows, dtype=np.int64)
    cols = np.asarray(cols, dtype=np.int64)
    vals = np.asarray(vals, dtype=np.float32)

    # --- window assignment: zigzag deal by degree ---
    deg = np.bincount(rows, minlength=N_pad)
    order = np.argsort(-deg, kind="stable")
    win_of = np.empty(N_pad, dtype=np.int64)
    fwd = np.arange(nwin)
    for r32 in range(32):
        seg = order[r32 * nwin:(r32 + 1) * nwin]
        win_of[seg] = fwd if (r32 % 2 == 0) else fwd[::-1]

    # slot formula needs perm; perm needs slot_of; iterate after repair.
    def perm_from(win_of, slot_of):
        return (win_of // WPT) * 128 + (win_of % WPT) * 32 + slot_of

    ids = np.arange(N_pad, dtype=np.int64)
    slot_arr = (ids // BLK) * BLK + (ids % 128) * T + (ids % BLK) // 128

    # --- per-(window, range) cell balance + repair ---
    # range of an edge depends on slot(perm[col]) which depends on win_of of
    # the SOURCE node. To keep this tractable we fix source positions first
    # (they're determined by win_of too). Iterate: compute, repair dest side.
    B = [2] * NR
    for _repair_round in range(30):
        slot_of = np.empty(N_pad, dtype=np.int64)
        # order within window: stable by node id
        ordw = np.lexsort((ids, win_of))
        slot_of[ordw] = np.arange(N_pad) % 32
        perm = perm_from(win_of, slot_of)
        ec = slot_arr[perm[cols]]
        erange = ec // range_len
        cell = win_of[rows] * NR + erange
        cellcnt = np.bincount(cell, minlength=nwin * NR).reshape(nwin, NR)
        cap = np.array([128 * b for b in B])
        over = cellcnt > cap[None, :]
        if not over.any():
            break
        # swap repair: for each violating (w, r), swap a high-deg_r row of w
        # with a low-deg_r row of the most-slack window w2 (same range r).
        ow, orr = np.nonzero(over)
        fixed_any = False
        for w, r in zip(ow[:64], orr[:64]):
            slack = cap[r] - cellcnt[:, r]
            w2 = int(np.argmax(slack))
            if slack[w2] <= 8:
                continue
            rows_w = np.nonzero(win_of == w)[0]
            rows_w2 = np.nonzero(win_of == w2)[0]
            # deg into range r ~ proxied by total degree (sources uniform)
            a = rows_w[np.argmax(deg[rows_w])]
            b2 = rows_w2[np.argmin(deg[rows_w2])]
            win_of[a], win_of[b2] = w2, w
            fixed_any = True
        if not fixed_any:
            # give up on swaps: bump budget of the worst range
            r = int(np.argmax(cellcnt.max(axis=0) / cap))
            B[r] += 1
    else:
        raise RuntimeError("cell balancing failed")

    NCHT, soff, batches = _chunk_layout(T, TB, B)
    Bar = np.array(B)
    soffar = np.array(soff)

    # --- final edge placement ---
    er = perm[rows]
    ec = slot_arr[perm[cols]]
    erange = ec // range_len
    erel = (ec - erange * range_len).astype(np.int16)
    ewin = (er // 128) * WPT + (er % 128) // 32
    eiw = er % 32
    cellid = ewin * NR + erange
    eord = np.argsort(cellid, kind="stable")
    cnt = np.bincount(cellid[eord], minlength=nwin * NR)
    off = np.zeros(nwin * NR + 1, dtype=np.int64)
    np.cumsum(cnt, out=off[1:])
    pos = np.arange(E, dtype=np.int64) - off[cellid[eord]]
    k = pos // 128
    lane = pos % 128
    assert (k < Bar[erange[eord]]).all(), "cell overflow after repair"

    es = eord  # sorted edge view
    ew = ewin[es]
    err_ = erange[es]
    core = ew // (T * WPT)
    t_of = (ew % (T * WPT)) // WPT
    wi = ew % WPT
    s_tile = soffar[err_] + wi * Bar[err_] + k      # in-tile chunk slot

    # --- per-core vals / seq arrays ---
    one_f8 = np.float32(1.0).astype(F8NP).view(np.uint8)
    gvals = np.zeros((n_cores, 128, T * NCHT), dtype=np.float32)
    gseq = np.zeros((n_cores, 128, T * NCHT * 32), dtype=np.uint8)
    colT = t_of * NCHT + s_tile
    gvals[core, lane, colT] = vals[es]
    gseq[core, lane, colT * 32 + eiw[es]] = one_f8

    # --- gather idx arrays (wrapped-16 int16), call order (batch, range) ---
    # logical j within call (b, r): ((t-t0)*WPT + wi)*B[r]*128 + k*128 + lane
    IDXW = T * NCHT * 8  # int16 elems per partition
    gidx = np.zeros((n_cores, 128, IDXW), dtype=np.int16)
    # per-call free offsets
    call_off = {}
    o = 0
    for bi, (t0, tb) in enumerate(batches):
        for r in range(NR):
            n_call = tb * WPT * B[r] * 128
            call_off[(bi, r)] = (o, n_call)
            o += n_call // 16
    assert o == IDXW

    bi_of = t_of // TB  # valid because batches are uniform TB except last
    t0_of = np.minimum(bi_of * TB, T)  # start tile of batch
    jj = (((t_of - t0_of) * WPT + wi) * Bar[err_] + k) * 128 + lane
    off_tab = np.zeros((len(batches), NR), dtype=np.int64)
    for (bi, r), (oo, _n) in call_off.items():
        off_tab[bi, r] = oo
    co = off_tab[bi_of, err_]
    s16 = jj // 16
    p16 = jj % 16
    # scatter, replicated 8x across 16-partition groups
    erel_s = erel[es]
    for rep in range(8):
        gidx[core, p16 + 16 * rep, co + s16] = erel_s
    # note: pad lanes stay 0 -> they gather row base_r+0 (valid, masked by seq=0)

    # --- ego in block layout ---
    ego_g = np.zeros((N_pad, D), dtype=np.float32)
    ego_g[slot_arr[perm[np.arange(n_nodes)]]] = np.asarray(ego, dtype=np.float32)
    ego_l = ego_g.reshape(n_cores, 128, T, D).reshape(n_cores, 128, T * D)

    cfg = dict(n_cores=n_cores, T=T, TB=TB, B=tuple(B), NCHT=NCHT,
               N_pad=N_pad, range_len=range_len)
    inmaps = []
    for c in range(n_cores):
        inmaps.append({
            "ego_g": ego_g,
            "ego_l": np.ascontiguousarray(ego_l[c]),
            "gidx": np.ascontiguousarray(gidx[c]),
            "gvals": np.ascontiguousarray(gvals[c]),
            "gseq": np.ascontiguousarray(gseq[c]).view(F8NP),
        })
    return cfg, inmaps, perm


def post_block(cfg, arrs):
    T = cfg["T"]
    blocks = [a.reshape(128, T, D).transpose(1, 0, 2).reshape(T * 128, D)
              for a in arrs]
    return np.concatenate(blocks, axis=0)


# ----------------------------------------------------------------------------
# device program
# ----------------------------------------------------------------------------

def build(nc, cfg):
    n_cores, T, TB = cfg["n_cores"], cfg["T"], cfg["TB"]
    B, NCHT, N_pad, range_len = list(cfg["B"]), cfg["NCHT"], cfg["N_pad"], cfg["range_len"]
    NCHT2, soff, batches = _chunk_layout(T, TB, B)
    assert NCHT2 == NCHT
    IDXW = T * NCHT * 8

    ego_g = nc.dram_tensor("ego_g", [N_pad, D], F32, kind="ExternalInput").ap()
    ego_l = nc.dram_tensor("ego_l", [128, T * D], F32, kind="ExternalInput").ap()
    gidx = nc.dram_tensor("gidx", [128, IDXW], I16, kind="ExternalInput").ap()
    gvals = nc.dram_tensor("gvals", [128, T * NCHT], F32, kind="ExternalInput").ap()
    gseq = nc.dram_tensor("gseq", [128, T * NCHT * 32], F8, kind="ExternalInput").ap()
    outs = [nc.dram_tensor(f"out{i}", [128, T * D], F32, kind="ExternalOutput").ap()
            for i in (1, 2, 3)]
    outsum = nc.dram_tensor("outsum", [128, T * D], F32, kind="ExternalOutput").ap()
    aspace = "Shared" if n_cores > 4 else "Local"
    h1g = nc.dram_tensor("h1g", [N_pad, D], F32, kind="Internal",
                         addr_space=aspace).ap()
    h2g = nc.dram_tensor("h2g", [N_pad, D], F32, kind="Internal",
                         addr_space=aspace).ap()
    agin = [nc.dram_tensor(f"agin{i}", [128, T * D], F32, kind="Internal").ap()
            for i in (1, 2)]

    hsrc = [ego_g, h1g, h2g]
    hdst = [h1g, h2g]
    groups = [list(range(n_cores))]

    # per-batch tables
    def call_tables(tb):
        """chunk-start per range and idx free-offsets within the batch."""
        cstart = []
        c = 0
        for r in range(NR):
            cstart.append(c)
            c += tb * WPT * B[r]
        return cstart, c  # c == tb*NCHT

    with tile.TileContext(nc) as tc:
        with (
            tc.tile_pool(name="persist", bufs=1) as pp,
            tc.tile_pool(name="idxp", bufs=2) as idxpool,
            tc.tile_pool(name="gatp", bufs=2) as gatpool,
            tc.tile_pool(name="seqp", bufs=3) as seqpool,
            tc.tile_pool(name="smatp", bufs=3) as smatpool,
            tc.tile_pool(name="ytp", bufs=4) as ypool,
            tc.tile_pool(name="psum", bufs=8, space="PSUM") as psump,
        ):
            hsum = pp.tile([128, T * D], F32, tag="hsum")
            vals_sb = pp.tile([128, T * NCHT], F32, tag="vals")
            nc.sync.dma_start(out=hsum[:], in_=ego_l)
            nc.sync.dma_start(out=vals_sb[:], in_=gvals)

            for L in range(3):
                idx_free = 0  # running offset into gidx (per batch)
                for bi, (t0, tb) in enumerate(batches):
                    cstart, ncc = call_tables(tb)
                    wbi = tb * NCHT * 8  # idx int16 elems this batch
                    idx_sb = idxpool.tile([128, TB * NCHT * 8], I16, tag="idx")
                    nc.sync.dma_start(
                        out=idx_sb[:, :wbi],
                        in_=gidx[:, idx_free:idx_free + wbi])
                    gat = gatpool.tile([128, TB * NCHT, D], F32, tag="gat")
                    ifree = 0
                    for r in range(NR):
                        n_call = tb * WPT * B[r] * 128
                        base = r * range_len
                        rlen = min(range_len, N_pad - base)
                        nc.gpsimd.dma_gather(
                            out_ap=gat[:, cstart[r]:cstart[r] + tb * WPT * B[r], :],
                            in_ap=hsrc[L][base:base + rlen, :],
                            idxs_ap=idx_sb[:, ifree:ifree + n_call // 16],
                            num_idxs=n_call,
                            num_idxs_reg=n_call,
                            elem_size=D,
                            single_packet=False,
                        )
                        ifree += n_call // 16
                    idx_free += wbi

                    for ti in range(tb):
                        t = t0 + ti
                        seq_sb = seqpool.tile([128, NCHT, 32], F8, tag="seq")
                        nc.sync.dma_start(
                            out=seq_sb[:],
                            in_=gseq[:, t * NCHT * 32:(t + 1) * NCHT * 32])
                        smat = smatpool.tile([128, NCHT, 32], F32, tag="smat")
                        nc.vector.tensor_tensor(
                            out=smat[:],
                            in0=seq_sb[:],
                            in1=vals_sb[:, t * NCHT:(t + 1) * NCHT]
                                .to_broadcast([128, NCHT, 32]),
                            op=mybir.AluOpType.mult,
                        )
                        y_t = ypool.tile([128, D], F32, tag="yt")
                        for wi in range(WPT):
                            ps = psump.tile([32, D], F32, tag="ps")
                            nmm = sum(B)
                            mi = 0
                            for r in range(NR):
                                for k in range(B[r]):
                                    s = soff[r] + wi * B[r] + k
                                    cc = cstart[r] + (ti * WPT + wi) * B[r] + k
                                    nc.tensor.matmul(
                                        out=ps[:],
                                        lhsT=smat[:, s, :],
                                        rhs=gat[:, cc, :],
                                        start=(mi == 0),
                                        stop=(mi == nmm - 1),
                                    )
                                    mi += 1
                            rs = slice(32 * wi, 32 * wi + 32)
                            nc.scalar.copy(out=y_t[rs, :], in_=ps[:])
                            nc.vector.tensor_add(
                                out=hsum[rs, t * D:(t + 1) * D],
                                in0=hsum[rs, t * D:(t + 1) * D],
                                in1=ps[:],
                            )
                        ytgt = agin[L] if L < 2 else outs[2]
                        nc.sync.dma_start(
                            out=ytgt[:, t * D:(t + 1) * D], in_=y_t[:])

                if L < 2:
                    nc.sync.dma_start(out=outs[L], in_=agin[L])
                    nc.gpsimd.collective_compute(
                        "AllGather",
                        mybir.AluOpType.bypass,
                        replica_groups=groups,
                        ins=[agin[L]],
                        outs=[hdst[L]],
                    )
            nc.sync.dma_start(out=outsum, in_=hsum[:])
    return nc


def make_nc(cfg, trn_type="TRN2"):
    nc = bacc.Bacc(trn_type, target_bir_lowering=False, debug=False,
                   num_devices=cfg["n_cores"])
    build(nc, cfg)
    nc.compile()
    return nc


# ----------------------------------------------------------------------------
# harness entry point: kernel(**inputs) -> full outputs
# ----------------------------------------------------------------------------

from concourse import bass_utils as _bass_utils

_N_USERS = 100000
_N_ITEMS = 50000
_N_NODES = _N_USERS + _N_ITEMS + 1
_N_CORES = 8
_T = 147

_CACHE = {}


def kernel(user_emb, item_emb, vals, rows, cols):
    user_emb = np.asarray(user_emb, dtype=np.float32)
    item_emb = np.asarray(item_emb, dtype=np.float32)
    vals = np.asarray(vals, dtype=np.float32)
    rows = np.asarray(rows)
    cols = np.asarray(cols)

    ego = np.concatenate([user_emb, item_emb], axis=0)
    cfg, inmaps, perm = prep(_N_CORES, _T, _N_NODES, ego, vals, rows, cols)
    key = (cfg["B"],)
    if key not in _CACHE:
        _CACHE[key] = make_nc(cfg)
    nc = _CACHE[key]

    res = _bass_utils.run_bass_kernel_spmd(
        nc, inmaps, core_ids=list(range(_N_CORES)))

    pi = perm[np.arange(_N_NODES)]

    def get(name):
        full = post_block(cfg, [res.results[c][name]
                                for c in range(_N_CORES)])
        return np.ascontiguousarray(full[pi])

    h1, h2, h3, hsum = get("out1"), get("out2"), get("out3"), get("outsum")
    return (hsum, np.ascontiguousarray(ego[:_N_NODES]), h1, h2, h3)



# revision 2
# speedup vs baseline: 2.6504x; 2.6504x over previous
"""GCN teacher 3-layer SpMM kernel for 8 trn2 NeuronCores (self-contained).

Strategy: row-shard nodes across 8 cores (147 tiles x 128 rows each, padded,
with a degree-balancing row permutation); per layer, per-edge dma_gather of
fp16 h[col] rows (256B padded rows; int16 range-relative indices over 5
sub-32768-slot ranges) spread over 4 SWDGE queues for parallel descriptor
generation; scatter-add via one-hot PE matmuls in fp16 (128-edge chunks into
the tile's 128-row window, fp8 one-hot from host scaled by fp16 vals on DVE),
PSUM fp32 accumulation per tile, fp16 AllGather republishes h between layers.
"""
import sys as _sys
for _p in ("/opt/trn_rl_repo",):
    if _p not in _sys.path:
        _sys.path.insert(0, _p)

import math
import numpy as np
import ml_dtypes

import concourse.bass as bass
import concourse.bacc as bacc
import concourse.tile as tile
from concourse import mybir

F32 = mybir.dt.float32
F16 = mybir.dt.float16
I16 = mybir.dt.int16
F8 = mybir.dt.float8e4
F8NP = ml_dtypes.float8_e4m3

D = 64
DP = 128         # padded fp16 row width (256B stride for dma_gather)
NR = 5           # source ranges
NQ = 4           # SWDGE queues (Q7 core pairs) for gather desc-gen


def _chunk_layout(T, TB, B):
    """Static call/chunk layout tables shared by prep() and build()."""
    NCHT = sum(B)
    soff = [sum(B[:r]) for r in range(NR)]  # in-tile chunk offset per range
    batches = []
    t0 = 0
    while t0 < T:
        tb = min(TB, T - t0)
        batches.append((t0, tb))
        t0 += tb
    return NCHT, soff, batches


def prep(n_cores, T, n_nodes, ego, vals, rows, cols, TB=4):
    N_pad = n_cores * T * 128
    BLK = T * 128
    nwin = N_pad // 128          # 128-row dest windows == tiles
    E = rows.shape[0]
    range_len = math.ceil(N_pad / NR)
    assert range_len <= 32768

    rows = np.asarray(rows, dtype=np.int64)
    cols = np.asarray(cols, dtype=np.int64)
    vals = np.asarray(vals, dtype=np.float32)

    # --- window (tile) assignment: zigzag deal by degree ---
    deg = np.bincount(rows, minlength=N_pad)
    order = np.argsort(-deg, kind="stable")
    win_of = np.empty(N_pad, dtype=np.int64)
    fwd = np.arange(nwin)
    for r128 in range(128):
        seg = order[r128 * nwin:(r128 + 1) * nwin]
        win_of[seg] = fwd if (r128 % 2 == 0) else fwd[::-1]

    ids = np.arange(N_pad, dtype=np.int64)
    # logical position -> physical DRAM row (partition-major within block)
    slot_arr = (ids // BLK) * BLK + (ids % 128) * T + (ids % BLK) // 128

    # --- per-(window, range) cell balance + repair ---
    B = [7] * NR
    for _repair_round in range(40):
        slot_of = np.empty(N_pad, dtype=np.int64)
        ordw = np.lexsort((ids, win_of))
        slot_of[ordw] = np.arange(N_pad) % 128
        perm = win_of * 128 + slot_of
        ec = slot_arr[perm[cols]]
        erange = ec // range_len
        cell = win_of[rows] * NR + erange
        cellcnt = np.bincount(cell, minlength=nwin * NR).reshape(nwin, NR)
        cap = np.array([128 * b for b in B])
        over = cellcnt > cap[None, :]
        if not over.any():
            break
        ow, orr = np.nonzero(over)
        fixed_any = False
        for w, r in zip(ow[:64], orr[:64]):
            slack = cap[r] - cellcnt[:, r]
            w2 = int(np.argmax(slack))
            if slack[w2] <= 16:
                continue
            rows_w = np.nonzero(win_of == w)[0]
            rows_w2 = np.nonzero(win_of == w2)[0]
            a = rows_w[np.argmax(deg[rows_w])]
            b2 = rows_w2[np.argmin(deg[rows_w2])]
            win_of[a], win_of[b2] = w2, w
            fixed_any = True
        if not fixed_any:
            r = int(np.argmax(cellcnt.max(axis=0) / cap))
            B[r] += 1
    else:
        raise RuntimeError("cell balancing failed")

    NCHT, soff, batches = _chunk_layout(T, TB, B)
    Bar = np.array(B)
    soffar = np.array(soff)

    # --- final edge placement ---
    er = perm[rows]
    ec = slot_arr[perm[cols]]
    erange = ec // range_len
    erel = (ec - erange * range_len).astype(np.int16)
    ewin = er // 128
    eiw = er % 128
    cellid = ewin * NR + erange
    eord = np.argsort(cellid, kind="stable")
    cnt = np.bincount(cellid[eord], minlength=nwin * NR)
    off = np.zeros(nwin * NR + 1, dtype=np.int64)
    np.cumsum(cnt, out=off[1:])
    pos = np.arange(E, dtype=np.int64) - off[cellid[eord]]
    k = pos // 128
    lane = pos % 128
    assert (k < Bar[erange[eord]]).all(), "cell overflow after repair"

    es = eord  # sorted edge view
    err_ = erange[es]
    core = ewin[es] // T
    t_of = ewin[es] % T
    s_tile = soffar[err_] + k      # in-tile chunk slot

    # --- per-core vals / seq arrays ---
    one_f8 = np.float32(1.0).astype(F8NP).view(np.uint8)
    gvals = np.zeros((n_cores, 128, T * NCHT), dtype=np.float16)
    gseq = np.zeros((n_cores, 128, T * NCHT * DP), dtype=np.uint8)
    colT = t_of * NCHT + s_tile
    gvals[core, lane, colT] = vals[es].astype(np.float16)
    gseq[core, lane, colT * DP + eiw[es]] = one_f8

    # --- gather idx arrays (wrapped-16 int16), call order (batch, range) ---
    # logical j within call (b, r): ((t-t0)*B[r] + k)*128 + lane
    IDXW = T * NCHT * 8  # int16 elems per partition
    gidx = np.zeros((n_cores, 128, IDXW), dtype=np.int16)
    call_off = {}
    o = 0
    for bi, (t0, tb) in enumerate(batches):
        for r in range(NR):
            n_call = tb * B[r] * 128
            call_off[(bi, r)] = (o, n_call)
            o += n_call // 16
    assert o == IDXW

    bi_of = t_of // TB
    t0_of = bi_of * TB
    jj = ((t_of - t0_of) * Bar[err_] + k) * 128 + lane
    off_tab = np.zeros((len(batches), NR), dtype=np.int64)
    for (bi, r), (oo, _n) in call_off.items():
        off_tab[bi, r] = oo
    co = off_tab[bi_of, err_]
    s16 = jj // 16
    p16 = jj % 16
    erel_s = erel[es]
    for rep in range(8):
        gidx[core, p16 + 16 * rep, co + s16] = erel_s
    # note: pad lanes stay 0 -> they gather row base_r+0 (valid, seq=0 masks)

    # --- ego in padded fp16 gather layout + fp32 block layout for hsum ---
    phys = slot_arr[perm[np.arange(n_nodes)]]
    ego32 = np.asarray(ego, dtype=np.float32)
    ego_g = np.zeros((N_pad, DP), dtype=np.float16)
    ego_g[phys, :D] = ego32.astype(np.float16)
    ego_g32 = np.zeros((N_pad, D), dtype=np.float32)
    ego_g32[phys] = ego32
    ego_l = ego_g32.reshape(n_cores, 128, T, D).reshape(n_cores, 128, T * D)

    cfg = dict(n_cores=n_cores, T=T, TB=TB, B=tuple(B), NCHT=NCHT,
               N_pad=N_pad, range_len=range_len)
    inmaps = []
    for c in range(n_cores):
        inmaps.append({
            "ego_g": ego_g,
            "ego_l": np.ascontiguousarray(ego_l[c]),
            "gidx": np.ascontiguousarray(gidx[c]),
            "gvals": np.ascontiguousarray(gvals[c]),
            "gseq": np.ascontiguousarray(gseq[c]).view(F8NP),
        })
    return cfg, inmaps, perm


def post_block(cfg, arrs):
    T = cfg["T"]
    blocks = [np.asarray(a, dtype=np.float32).reshape(128, T, D)
              .transpose(1, 0, 2).reshape(T * 128, D) for a in arrs]
    return np.concatenate(blocks, axis=0)


# ----------------------------------------------------------------------------
# device program
# ----------------------------------------------------------------------------

def build(nc, cfg):
    n_cores, T, TB = cfg["n_cores"], cfg["T"], cfg["TB"]
    B, NCHT, N_pad, range_len = (list(cfg["B"]), cfg["NCHT"], cfg["N_pad"],
                                 cfg["range_len"])
    NCHT2, soff, batches = _chunk_layout(T, TB, B)
    assert NCHT2 == NCHT
    IDXW = T * NCHT * 8

    ego_g = nc.dram_tensor("ego_g", [N_pad, DP], F16, kind="ExternalInput").ap()
    ego_l = nc.dram_tensor("ego_l", [128, T * D], F32, kind="ExternalInput").ap()
    gidx = nc.dram_tensor("gidx", [128, IDXW], I16, kind="ExternalInput").ap()
    gvals = nc.dram_tensor("gvals", [128, T * NCHT], F16, kind="ExternalInput").ap()
    gseq = nc.dram_tensor("gseq", [128, T * NCHT * DP], F8, kind="ExternalInput").ap()
    outs = [nc.dram_tensor(f"out{i}", [128, T, D], F16, kind="ExternalOutput").ap()
            for i in (1, 2, 3)]
    outsum = nc.dram_tensor("outsum", [128, T * D], F32, kind="ExternalOutput").ap()
    aspace = "Shared" if n_cores > 4 else "Local"
    h1g = nc.dram_tensor("h1g", [N_pad, DP], F16, kind="Internal",
                         addr_space=aspace).ap()
    h2g = nc.dram_tensor("h2g", [N_pad, DP], F16, kind="Internal",
                         addr_space=aspace).ap()
    agin = [nc.dram_tensor(f"agin{i}", [128, T, DP], F16, kind="Internal").ap()
            for i in (1, 2)]

    hsrc = [ego_g, h1g, h2g]
    hdst = [h1g, h2g]
    groups = [list(range(n_cores))]

    def call_tables(tb):
        """chunk-start per range within the batch's gat tile."""
        cstart = []
        c = 0
        for r in range(NR):
            cstart.append(c)
            c += tb * B[r]
        return cstart, c  # c == tb*NCHT

    qctr = 0
    with tile.TileContext(nc) as tc:
        with (
            tc.tile_pool(name="persist", bufs=1) as pp,
            tc.tile_pool(name="idxp", bufs=2) as idxpool,
            tc.tile_pool(name="gatp", bufs=2) as gatpool,
            tc.tile_pool(name="seqp", bufs=3) as seqpool,
            tc.tile_pool(name="smatp", bufs=3) as smatpool,
            tc.tile_pool(name="ytp", bufs=4) as ypool,
            tc.tile_pool(name="psum", bufs=8, space="PSUM") as psump,
        ):
            hsum = pp.tile([128, T * D], F32, tag="hsum")
            vals_sb = pp.tile([128, T * NCHT], F16, tag="vals")
            nc.sync.dma_start(out=hsum[:], in_=ego_l)
            nc.sync.dma_start(out=vals_sb[:], in_=gvals)

            for L in range(3):
                idx_free = 0  # running offset into gidx (per batch)
                for bi, (t0, tb) in enumerate(batches):
                    cstart, ncc = call_tables(tb)
                    wbi = tb * NCHT * 8  # idx int16 elems this batch
                    idx_sb = idxpool.tile([128, TB * NCHT * 8], I16, tag="idx")
                    nc.sync.dma_start(
                        out=idx_sb[:, :wbi],
                        in_=gidx[:, idx_free:idx_free + wbi])
                    gat = gatpool.tile([128, TB * NCHT, DP], F16, tag="gat")
                    ifree = 0
                    for r in range(NR):
                        n_call = tb * B[r] * 128
                        base = r * range_len
                        rlen = min(range_len, N_pad - base)
                        nc.gpsimd.dma_gather(
                            out_ap=gat[:, cstart[r]:cstart[r] + tb * B[r], :],
                            in_ap=hsrc[L][base:base + rlen, :],
                            idxs_ap=idx_sb[:, ifree:ifree + n_call // 16],
                            num_idxs=n_call,
                            num_idxs_reg=n_call,
                            elem_size=DP,
                            single_packet=False,
                            queue_num=qctr % NQ,
                        )
                        qctr += 1
                        ifree += n_call // 16
                    idx_free += wbi

                    for ti in range(tb):
                        t = t0 + ti
                        seq_sb = seqpool.tile([128, NCHT, DP], F8, tag="seq")
                        nc.sync.dma_start(
                            out=seq_sb[:],
                            in_=gseq[:, t * NCHT * DP:(t + 1) * NCHT * DP])
                        smat = smatpool.tile([128, NCHT, DP], F16, tag="smat")
                        nc.vector.tensor_tensor(
                            out=smat[:],
                            in0=seq_sb[:],
                            in1=vals_sb[:, t * NCHT:(t + 1) * NCHT]
                                .to_broadcast([128, NCHT, DP]),
                            op=mybir.AluOpType.mult,
                        )
                        ps = psump.tile([128, D], F32, tag="ps")
                        mi = 0
                        for r in range(NR):
                            for k in range(B[r]):
                                s = soff[r] + k
                                cc = cstart[r] + ti * B[r] + k
                                nc.tensor.matmul(
                                    out=ps[:],
                                    lhsT=smat[:, s, :],
                                    rhs=gat[:, cc, 0:D],
                                    start=(mi == 0),
                                    stop=(mi == NCHT - 1),
                                )
                                mi += 1
                        y16 = ypool.tile([128, D], F16, tag="yt")
                        nc.scalar.copy(out=y16[:], in_=ps[:])
                        nc.vector.tensor_add(
                            out=hsum[:, t * D:(t + 1) * D],
                            in0=hsum[:, t * D:(t + 1) * D],
                            in1=ps[:],
                        )
                        if L < 2:
                            nc.sync.dma_start(
                                out=agin[L][:, t, 0:D], in_=y16[:])
                        else:
                            nc.sync.dma_start(
                                out=outs[2][:, t, :], in_=y16[:])

                if L < 2:
                    nc.sync.dma_start(out=outs[L], in_=agin[L][:, :, 0:D])
                    nc.gpsimd.collective_compute(
                        "AllGather",
                        mybir.AluOpType.bypass,
                        replica_groups=groups,
                        ins=[agin[L]],
                        outs=[hdst[L]],
                    )
            nc.sync.dma_start(out=outsum, in_=hsum[:])
    return nc


def make_nc(cfg, trn_type="TRN2"):
    nc = bacc.Bacc(trn_type, target_bir_lowering=False, debug=False,
                   num_devices=cfg["n_cores"], num_swdge_queues=NQ)
    build(nc, cfg)
    nc.compile()
    return nc


# ----------------------------------------------------------------------------
# harness entry point: kernel(**inputs) -> full outputs
# ----------------------------------------------------------------------------

from concourse import bass_utils as _bass_utils

_N_USERS = 100000
_N_ITEMS = 50000
_N_NODES = _N_USERS + _N_ITEMS + 1
_N_CORES = 8
_T = 147

_CACHE = {}


def kernel(user_emb, item_emb, vals, rows, cols):
    user_emb = np.asarray(user_emb, dtype=np.float32)
    item_emb = np.asarray(item_emb, dtype=np.float32)
    vals = np.asarray(vals, dtype=np.float32)
    rows = np.asarray(rows)
    cols = np.asarray(cols)

    ego = np.concatenate([user_emb, item_emb], axis=0)
    cfg, inmaps, perm = prep(_N_CORES, _T, _N_NODES, ego, vals, rows, cols)
    key = (cfg["B"],)
    if key not in _CACHE:
        _CACHE[key] = make_nc(cfg)
    nc = _CACHE[key]

    res = _bass_utils.run_bass_kernel_spmd(
        nc, inmaps, core_ids=list(range(_N_CORES)))

    pi = perm[np.arange(_N_NODES)]

    def get(name):
        full = post_block(cfg, [res.results[c][name]
                                for c in range(_N_CORES)])
        return np.ascontiguousarray(full[pi])

    h1, h2, h3, hsum = get("out1"), get("out2"), get("out3"), get("outsum")
    return (hsum, np.ascontiguousarray(ego[:_N_NODES]), h1, h2, h3)


# revision 8
# speedup vs baseline: 3.1445x; 1.1865x over previous
"""GCN teacher 3-layer SpMM kernel for 8 trn2 NeuronCores (self-contained).

Strategy: row-shard nodes across 8 cores (147 tiles x 128 rows each, padded,
with a degree-balancing row permutation); per layer, per-edge dma_gather of
fp16 h[col] rows (256B padded rows; int16 range-relative indices over 5
sub-32768-slot ranges) spread over 4 SWDGE queues for parallel descriptor
generation; scatter-add via one-hot PE matmuls in fp16 (128-edge chunks into
the tile's 128-row window, fp8 one-hot from host scaled by fp16 vals on DVE),
PSUM fp32 accumulation per tile, fp16 AllGather republishes h between layers.
"""
import sys as _sys
for _p in ("/opt/trn_rl_repo",):
    if _p not in _sys.path:
        _sys.path.insert(0, _p)

import math
import numpy as np
import ml_dtypes

import concourse.bass as bass
import concourse.bacc as bacc
import concourse.tile as tile
from concourse import mybir

F32 = mybir.dt.float32
F16 = mybir.dt.float16
I16 = mybir.dt.int16
F8 = mybir.dt.float8e4
F8NP = ml_dtypes.float8_e4m3

D = 64
DP = 128         # padded fp16 row width (256B stride for dma_gather)
NR = 5           # source ranges
NQ = 4           # SWDGE queues (Q7 core pairs) for gather desc-gen


def _chunk_layout(T, TB, B):
    """Static call/chunk layout tables shared by prep() and build()."""
    NCHT = sum(B)
    soff = [sum(B[:r]) for r in range(NR)]  # in-tile chunk offset per range
    batches = []
    t0 = 0
    while t0 < T:
        tb = min(TB, T - t0)
        batches.append((t0, tb))
        t0 += tb
    return NCHT, soff, batches


def prep(n_cores, T, n_nodes, ego, vals, rows, cols, TB=4):
    N_pad = n_cores * T * 128
    BLK = T * 128
    nwin = N_pad // 128          # 128-row dest windows == tiles
    E = rows.shape[0]
    range_len = math.ceil(N_pad / NR)
    assert range_len <= 32768

    rows = np.asarray(rows, dtype=np.int64)
    cols = np.asarray(cols, dtype=np.int64)
    vals = np.asarray(vals, dtype=np.float32)

    # --- window (tile) assignment: zigzag deal by degree ---
    deg = np.bincount(rows, minlength=N_pad)
    order = np.argsort(-deg, kind="stable")
    win_of = np.empty(N_pad, dtype=np.int64)
    fwd = np.arange(nwin)
    for r128 in range(128):
        seg = order[r128 * nwin:(r128 + 1) * nwin]
        win_of[seg] = fwd if (r128 % 2 == 0) else fwd[::-1]

    ids = np.arange(N_pad, dtype=np.int64)
    # logical position -> physical DRAM row (partition-major within block)
    slot_arr = (ids // BLK) * BLK + (ids % 128) * T + (ids % BLK) // 128

    # --- per-(window, range) cell balance + repair ---
    B = [7] * NR
    for _repair_round in range(40):
        slot_of = np.empty(N_pad, dtype=np.int64)
        ordw = np.lexsort((ids, win_of))
        slot_of[ordw] = np.arange(N_pad) % 128
        perm = win_of * 128 + slot_of
        ec = slot_arr[perm[cols]]
        erange = ec // range_len
        cell = win_of[rows] * NR + erange
        cellcnt = np.bincount(cell, minlength=nwin * NR).reshape(nwin, NR)
        cap = np.array([128 * b for b in B])
        over = cellcnt > cap[None, :]
        if not over.any():
            break
        ow, orr = np.nonzero(over)
        fixed_any = False
        for w, r in zip(ow[:64], orr[:64]):
            slack = cap[r] - cellcnt[:, r]
            w2 = int(np.argmax(slack))
            if slack[w2] <= 16:
                continue
            rows_w = np.nonzero(win_of == w)[0]
            rows_w2 = np.nonzero(win_of == w2)[0]
            a = rows_w[np.argmax(deg[rows_w])]
            b2 = rows_w2[np.argmin(deg[rows_w2])]
            win_of[a], win_of[b2] = w2, w
            fixed_any = True
        if not fixed_any:
            r = int(np.argmax(cellcnt.max(axis=0) / cap))
            B[r] += 1
    else:
        raise RuntimeError("cell balancing failed")

    NCHT, soff, batches = _chunk_layout(T, TB, B)
    Bar = np.array(B)
    soffar = np.array(soff)

    # --- final edge placement ---
    er = perm[rows]
    ec = slot_arr[perm[cols]]
    erange = ec // range_len
    erel = (ec - erange * range_len).astype(np.int16)
    ewin = er // 128
    eiw = er % 128
    cellid = ewin * NR + erange
    eord = np.argsort(cellid, kind="stable")
    cnt = np.bincount(cellid[eord], minlength=nwin * NR)
    off = np.zeros(nwin * NR + 1, dtype=np.int64)
    np.cumsum(cnt, out=off[1:])
    pos = np.arange(E, dtype=np.int64) - off[cellid[eord]]
    k = pos // 128
    lane = pos % 128
    assert (k < Bar[erange[eord]]).all(), "cell overflow after repair"

    es = eord  # sorted edge view
    err_ = erange[es]
    core = ewin[es] // T
    t_of = ewin[es] % T
    s_tile = soffar[err_] + k      # in-tile chunk slot

    # --- per-core vals / seq arrays ---
    one_f8 = np.float32(1.0).astype(F8NP).view(np.uint8)
    gvals = np.zeros((n_cores, 128, T * NCHT), dtype=np.float16)
    gseq = np.zeros((n_cores, 128, T * NCHT * DP), dtype=np.uint8)
    colT = t_of * NCHT + s_tile
    gvals[core, lane, colT] = vals[es].astype(np.float16)
    gseq[core, lane, colT * DP + eiw[es]] = one_f8

    # --- gather idx arrays (wrapped-16 int16), call order (batch, range) ---
    # logical j within call (b, r): ((t-t0)*B[r] + k)*128 + lane
    IDXW = T * NCHT * 8  # int16 elems per partition
    gidx = np.zeros((n_cores, 128, IDXW), dtype=np.int16)
    call_off = {}
    o = 0
    for bi, (t0, tb) in enumerate(batches):
        for r in range(NR):
            n_call = tb * B[r] * 128
            call_off[(bi, r)] = (o, n_call)
            o += n_call // 16
    assert o == IDXW

    bi_of = t_of // TB
    t0_of = bi_of * TB
    jj = ((t_of - t0_of) * Bar[err_] + k) * 128 + lane
    off_tab = np.zeros((len(batches), NR), dtype=np.int64)
    for (bi, r), (oo, _n) in call_off.items():
        off_tab[bi, r] = oo
    co = off_tab[bi_of, err_]
    s16 = jj // 16
    p16 = jj % 16
    erel_s = erel[es]
    for rep in range(8):
        gidx[core, p16 + 16 * rep, co + s16] = erel_s
    # note: pad lanes stay 0 -> they gather row base_r+0 (valid, seq=0 masks)

    # trailing pad positions per call -> -1 so ucode skips their descriptors
    # (safe: trimmed slots retain finite fp16 from an earlier write; their
    # smat coefficients are 0)
    TRIM = False
    if TRIM:
        nb = len(batches)
        maxj = np.full((n_cores, nb, NR), -1, dtype=np.int64)
        np.maximum.at(maxj, (core, bi_of, err_), jj)
        for c in range(n_cores):
            for bi, (t0, tb) in enumerate(batches):
                for r in range(NR):
                    oo, n_call = call_off[(bi, r)]
                    last = maxj[c, bi, r]
                    # flat j in (last, n_call) -> -1 (wrapped-16 layout)
                    j0 = last + 1
                    full0 = (j0 + 15) // 16     # first fully-unused column
                    gidx[c, :, oo + full0:oo + n_call // 16] = -1
                    for j in range(j0, min(full0 * 16, n_call)):
                        gidx[c, j % 16::16, oo + j // 16] = -1

    # --- layer-0 pre-gathered rhs (ego is known on host): batch-block layout
    # col within batch bi = cstart_bi[r] + (t-t0)*B[r] + k, cstart_bi[r] =
    # tb*soff[r]; batch block starts at t0*NCHT.
    ego16c = np.zeros((N_pad, D), dtype=np.float16)
    # (filled below once ego_g is built)
    tb_of = np.minimum(TB, T - t0_of)
    cc_of = tb_of * soffar[err_] + (t_of - t0_of) * Bar[err_] + k
    g0col = (t0_of * NCHT + cc_of)
    g0idx = np.zeros((n_cores, 128, T * NCHT), dtype=np.int64)
    g0idx[core, lane, g0col] = ec[es]

    # --- ego in padded fp16 gather layout + fp32 block layout for hsum ---
    phys = slot_arr[perm[np.arange(n_nodes)]]
    ego32 = np.asarray(ego, dtype=np.float32)
    ego16 = ego32.astype(np.float16)
    ego_g = np.zeros((N_pad, DP), dtype=np.float16)
    ego_g[phys, :D] = ego16
    ego_g32 = np.zeros((N_pad, D), dtype=np.float32)
    ego_g32[phys] = ego32
    ego_l = ego_g32.reshape(n_cores, 128, T, D).reshape(n_cores, 128, T * D)
    ego16c[phys] = ego16

    cfg = dict(n_cores=n_cores, T=T, TB=TB, B=tuple(B), NCHT=NCHT,
               N_pad=N_pad, range_len=range_len)
    inmaps = []
    for c in range(n_cores):
        gat0 = ego16c[g0idx[c]].reshape(128, T * NCHT * D)
        inmaps.append({
            "ego_l": np.ascontiguousarray(ego_l[c]),
            "gidx": np.ascontiguousarray(gidx[c]),
            "gvals": np.ascontiguousarray(gvals[c]),
            "gseq": np.ascontiguousarray(gseq[c]).view(F8NP),
            "gat0": np.ascontiguousarray(gat0),
        })
    return cfg, inmaps, perm


def post_block(cfg, arrs):
    T = cfg["T"]
    blocks = [np.asarray(a, dtype=np.float32).reshape(128, T, D)
              .transpose(1, 0, 2).reshape(T * 128, D) for a in arrs]
    return np.concatenate(blocks, axis=0)


# ----------------------------------------------------------------------------
# device program
# ----------------------------------------------------------------------------

def build(nc, cfg):
    n_cores, T, TB = cfg["n_cores"], cfg["T"], cfg["TB"]
    B, NCHT, N_pad, range_len = (list(cfg["B"]), cfg["NCHT"], cfg["N_pad"],
                                 cfg["range_len"])
    NCHT2, soff, batches = _chunk_layout(T, TB, B)
    assert NCHT2 == NCHT
    IDXW = T * NCHT * 8

    ego_l = nc.dram_tensor("ego_l", [128, T * D], F32, kind="ExternalInput").ap()
    gidx = nc.dram_tensor("gidx", [128, IDXW], I16, kind="ExternalInput").ap()
    gvals = nc.dram_tensor("gvals", [128, T * NCHT], F16, kind="ExternalInput").ap()
    gseq = nc.dram_tensor("gseq", [128, T * NCHT * DP], F8, kind="ExternalInput").ap()
    gat0 = nc.dram_tensor("gat0", [128, T * NCHT * D], F16, kind="ExternalInput").ap()
    outs = [nc.dram_tensor(f"out{i}", [128, T, D], F16, kind="ExternalOutput").ap()
            for i in (1, 2, 3)]
    outsum = nc.dram_tensor("outsum", [128, T * D], F32, kind="ExternalOutput").ap()
    aspace = "Shared" if n_cores > 4 else "Local"
    h1g = nc.dram_tensor("h1g", [N_pad, DP], F16, kind="Internal",
                         addr_space=aspace).ap()
    h2g = nc.dram_tensor("h2g", [N_pad, DP], F16, kind="Internal",
                         addr_space=aspace).ap()
    agin = [nc.dram_tensor(f"agin{i}", [128, T, DP], F16, kind="Internal").ap()
            for i in (1, 2)]

    hsrc = [None, h1g, h2g]
    hdst = [h1g, h2g]
    groups = [list(range(n_cores))]

    def call_tables(tb):
        """chunk-start per range within the batch's gat tile."""
        cstart = []
        c = 0
        for r in range(NR):
            cstart.append(c)
            c += tb * B[r]
        return cstart, c  # c == tb*NCHT

    qctr = 0
    with tile.TileContext(nc) as tc:
        with (
            tc.tile_pool(name="persist", bufs=1) as pp,
            tc.tile_pool(name="idxp", bufs=2) as idxpool,
            tc.tile_pool(name="gatp", bufs=2) as gatpool,
            tc.tile_pool(name="seqp", bufs=3) as seqpool,
            tc.tile_pool(name="smatp", bufs=3) as smatpool,
            tc.tile_pool(name="ytp", bufs=4) as ypool,
            tc.tile_pool(name="psum", bufs=8, space="PSUM") as psump,
        ):
            hsum = pp.tile([128, T * D], F32, tag="hsum")
            vals_sb = pp.tile([128, T * NCHT], F16, tag="vals")
            nc.sync.dma_start(out=hsum[:], in_=ego_l)
            nc.sync.dma_start(out=vals_sb[:], in_=gvals)

            for L in range(3):
                idx_free = 0  # running offset into gidx (per batch)
                for bi, (t0, tb) in enumerate(batches):
                    cstart, ncc = call_tables(tb)
                    gat = gatpool.tile([128, TB * NCHT, DP], F16, tag="gat")
                    if L == 0:
                        # layer-0 rhs pre-gathered on host (ego known)
                        nc.sync.dma_start(
                            out=gat[:, 0:ncc, 0:D],
                            in_=gat0[:, t0 * NCHT * D:(t0 + tb) * NCHT * D])
                        idx_free += tb * NCHT * 8
                    else:
                        wbi = tb * NCHT * 8  # idx int16 elems this batch
                        idx_sb = idxpool.tile([128, TB * NCHT * 8], I16,
                                              tag="idx")
                        nc.sync.dma_start(
                            out=idx_sb[:, :wbi],
                            in_=gidx[:, idx_free:idx_free + wbi])
                        ifree = 0
                        for r in range(NR):
                            n_call = tb * B[r] * 128
                            base = r * range_len
                            rlen = min(range_len, N_pad - base)
                            nc.gpsimd.dma_gather(
                                out_ap=gat[:, cstart[r]:cstart[r] + tb * B[r], :],
                                in_ap=hsrc[L][base:base + rlen, :],
                                idxs_ap=idx_sb[:, ifree:ifree + n_call // 16],
                                num_idxs=n_call,
                                num_idxs_reg=n_call,
                                elem_size=DP,
                                single_packet=False,
                                queue_num=qctr % NQ,
                            )
                            qctr += 1
                            ifree += n_call // 16
                        idx_free += wbi

                    for ti in range(tb):
                        t = t0 + ti
                        seq_sb = seqpool.tile([128, NCHT, DP], F8, tag="seq")
                        nc.sync.dma_start(
                            out=seq_sb[:],
                            in_=gseq[:, t * NCHT * DP:(t + 1) * NCHT * DP])
                        smat = smatpool.tile([128, NCHT, DP], F16, tag="smat")
                        nc.vector.tensor_tensor(
                            out=smat[:],
                            in0=seq_sb[:],
                            in1=vals_sb[:, t * NCHT:(t + 1) * NCHT]
                                .to_broadcast([128, NCHT, DP]),
                            op=mybir.AluOpType.mult,
                        )
                        ps = psump.tile([128, D], F32, tag="ps")
                        mi = 0
                        for r in range(NR):
                            for k in range(B[r]):
                                s = soff[r] + k
                                cc = cstart[r] + ti * B[r] + k
                                nc.tensor.matmul(
                                    out=ps[:],
                                    lhsT=smat[:, s, :],
                                    rhs=gat[:, cc, 0:D],
                                    start=(mi == 0),
                                    stop=(mi == NCHT - 1),
                                )
                                mi += 1
                        y16 = ypool.tile([128, D], F16, tag="yt")
                        nc.scalar.copy(out=y16[:], in_=ps[:])
                        nc.vector.tensor_add(
                            out=hsum[:, t * D:(t + 1) * D],
                            in0=hsum[:, t * D:(t + 1) * D],
                            in1=ps[:],
                        )
                        if L < 2:
                            nc.sync.dma_start(
                                out=agin[L][:, t, 0:D], in_=y16[:])
                        else:
                            nc.sync.dma_start(
                                out=outs[2][:, t, :], in_=y16[:])

                if L < 2:
                    nc.sync.dma_start(out=outs[L], in_=agin[L][:, :, 0:D])
                    nc.gpsimd.collective_compute(
                        "AllGather",
                        mybir.AluOpType.bypass,
                        replica_groups=groups,
                        ins=[agin[L]],
                        outs=[hdst[L]],
                    )
            nc.sync.dma_start(out=outsum, in_=hsum[:])
    return nc


def make_nc(cfg, trn_type="TRN2"):
    nc = bacc.Bacc(trn_type, target_bir_lowering=False, debug=False,
                   num_devices=cfg["n_cores"], num_swdge_queues=NQ)
    build(nc, cfg)
    nc.compile()
    return nc


# ----------------------------------------------------------------------------
# harness entry point: kernel(**inputs) -> full outputs
# ----------------------------------------------------------------------------

from concourse import bass_utils as _bass_utils

_N_USERS = 100000
_N_ITEMS = 50000
_N_NODES = _N_USERS + _N_ITEMS + 1
_N_CORES = 8
_T = 147

_CACHE = {}


def kernel(user_emb, item_emb, vals, rows, cols):
    user_emb = np.asarray(user_emb, dtype=np.float32)
    item_emb = np.asarray(item_emb, dtype=np.float32)
    vals = np.asarray(vals, dtype=np.float32)
    rows = np.asarray(rows)
    cols = np.asarray(cols)

    ego = np.concatenate([user_emb, item_emb], axis=0)
    cfg, inmaps, perm = prep(_N_CORES, _T, _N_NODES, ego, vals, rows, cols)
    key = (cfg["B"],)
    if key not in _CACHE:
        _CACHE[key] = make_nc(cfg)
    nc = _CACHE[key]

    res = _bass_utils.run_bass_kernel_spmd(
        nc, inmaps, core_ids=list(range(_N_CORES)))

    pi = perm[np.arange(_N_NODES)]

    def get(name):
        full = post_block(cfg, [res.results[c][name]
                                for c in range(_N_CORES)])
        return np.ascontiguousarray(full[pi])

    h1, h2, h3, hsum = get("out1"), get("out2"), get("out3"), get("outsum")
    return (hsum, np.ascontiguousarray(ego[:_N_NODES]), h1, h2, h3)
